# revision 13
# baseline (speedup 1.0000x reference)
"""Multi-head attention (dense transformer block) on 8 Trainium2 NeuronCores.

Sharding: one attention head per core (H=8 heads, 8 cores), both batch
elements on every core. Each core computes q/k/v projections for its head,
the full NxN softmax attention, and returns the 65-row augmented attention
output (64 V-dims + softmax-denominator row) per 512-query chunk. The host
applies the small output projection w_o (fp32), divides by the denominator,
and sums the 8 per-head partials (the only cross-core reduction; no
device-to-device communication).

Per-core pipeline (matmul inputs fp16, fp32 PSUM accumulation; all matmuls
at base partition 0 — tile_position row/col packing measured ~9x SLOWER on
HW, so it is deliberately not used):
  QKV    psA = [Wq|Wv] d-chunks @ X^T, psB = [Wk|Wv] @ X^T  (K=128, M=128)
         q -> qT[0:64], k -> kpack[0:64], v -> vtmp[64:128]
  V^T    transpose with half identity: out[128,64] = (vtmp rows 64:128)^T
  S^T    MM(K=64): kpack j-block stationary, qT chunk streaming; j-blocks
         2p,2p+1 fill one [128,1024] PSUM slab (2 banks)
  P      split across engines: ACT exp(0.125*S + BA) -> fp16, and DVE
         fast-exp2: int16(round(C1*S + C2)) bit-cast to fp16 (Schraudolph),
         one fused tensor_scalar op. C2 shifts the global exp scale so the
         int16 value is always positive; V is pre-scaled by 2^-G on the
         host so fp16 Ou can't overflow; the scale cancels exactly in the
         host-side division by the returned denominator row.
  Ou^T   [V^T | 2^-G ones] @ P^T accumulated over 32 j-blocks (M=65; row 64
         accumulates l = 2^-G * sum P). TWO independent PSUM accumulation
         chains, one consuming ACT-exp tiles and one DVE-exp tiles, so the
         in-order PV chain never stalls on the other engine's completions;
         the chains are summed during evacuation.
  out    [65,512] fp32 evac PSUM->SBUF (copy + add), DMA to DRAM
"""
import numpy as np
from contextlib import ExitStack

import concourse.bass as bass
import concourse.tile as tile
from concourse import bacc, mybir
from concourse.bass_utils import run_bass_kernel_spmd
from concourse.masks import make_identity

dt = mybir.dt

H = 8
HD = 64
D = 512
B = 2
N = 4096
NB = B * N          # 8192
NCH = NB // 512     # 16 chunks of 512 queries
NPAIR = 16          # 32 j-blocks of 128, processed as 16 slab-pairs

LOG2E = 1.4426950408889634
C1 = 0.125 * LOG2E * 1024.0
C2 = 14932.0
BA = (C2 - 15360.0) / 1024.0 * float(np.log(2.0))  # ACT-path logit offset
G = 4                                              # V pre-scale 2^-G

MM_DT = dt.float16
MM_NP = np.float16

# evenly spread DVE-exp pair assignments by count
DVE_SETS = {0: (), 4: (2, 6, 10, 14), 5: (1, 4, 7, 10, 13),
            6: (1, 4, 6, 9, 12, 14), 7: (1, 3, 5, 8, 10, 12, 14),
            8: (0, 2, 4, 6, 8, 10, 12, 14), 10: (0, 1, 3, 4, 6, 8, 9, 11, 12, 14)}


def _build(repeat=1, ndve=7, spair_bufs=3, pt_bufs=10, xt_bufs=8, ot_bufs=4):
    dve_set = set(DVE_SETS[ndve])
    nc = bacc.Bacc("TRN2", target_bir_lowering=False, debug=False, num_devices=8)
    xt = nc.dram_tensor("xt", [D, NB], MM_DT, kind="ExternalInput").ap()
    wa = nc.dram_tensor("wa", [D, 128], MM_DT, kind="ExternalInput").ap()
    wb = nc.dram_tensor("wb", [D, 128], MM_DT, kind="ExternalInput").ap()
    oub = nc.dram_tensor("oub", [NCH, HD + 1, 512], dt.float32,
                         kind="ExternalOutput").ap()

    with tile.TileContext(nc) as tc:
        with ExitStack() as ctx:
            const_p = ctx.enter_context(tc.tile_pool(name="const", bufs=1))
            persist = ctx.enter_context(tc.tile_pool(name="persist", bufs=1))
            xt_p = ctx.enter_context(tc.tile_pool(name="xtp", bufs=xt_bufs))
            pt_p = ctx.enter_context(tc.tile_pool(name="ptp", bufs=pt_bufs))
            ot_p = ctx.enter_context(tc.tile_pool(name="otp", bufs=ot_bufs))
            spool = ctx.enter_context(
                tc.tile_pool(name="spool", bufs=spair_bufs, space="PSUM"))
            opool = ctx.enter_context(
                tc.tile_pool(name="opool", bufs=2, space="PSUM"))

            ident = const_p.tile([128, 128], MM_DT, tag="ident")
            make_identity(nc, ident[:])
            actbias = const_p.tile([128, 1], dt.float32, tag="actbias")
            nc.vector.memset(actbias[:], BA)
            wa_s = const_p.tile([128, 4, 128], MM_DT, tag="wa")
            wb_s = const_p.tile([128, 4, 128], MM_DT, tag="wb")
            for d in range(4):
                nc.sync.dma_start(wa_s[:, d, :], wa[d * 128:(d + 1) * 128, :])
                nc.sync.dma_start(wb_s[:, d, :], wb[d * 128:(d + 1) * 128, :])

            # persistent activation layouts (see module docstring)
            qT = persist.tile([64, NB], MM_DT, tag="qT")
            kpack = persist.tile([64, NB], MM_DT, tag="kpack")
            vtmp = persist.tile([128, NB], MM_DT, tag="vtmp")
            vaug = [persist.tile([128, 32 * 65], MM_DT, tag=f"vaug{b}",
                                 name=f"vaug{b}") for b in range(B)]
            for b in range(B):
                nc.vector.memset(vaug[b][:], 2.0 ** (-G))
            nc.vector.memset(vtmp[0:64, :], 0.0)

            def body(_=None):
                # ---- QKV projection ----
                for ci in range(NCH):
                    c0 = ci * 512
                    xts = []
                    for d in range(4):
                        t = xt_p.tile([128, 512], MM_DT, tag="xt", name="xt_t")
                        nc.sync.dma_start(
                            t[:], xt[d * 128:(d + 1) * 128, c0:c0 + 512])
                        xts.append(t)
                    ps = spool.tile([128, 1024], dt.float32, tag="spair", name="ps_qkv")
                    for d in range(4):
                        nc.tensor.matmul(ps[:, 0:512], wa_s[:, d, :], xts[d][:],
                                         start=(d == 0), stop=(d == 3))
                    for d in range(4):
                        nc.tensor.matmul(ps[:, 512:1024], wb_s[:, d, :], xts[d][:],
                                         start=(d == 0), stop=(d == 3))
                    nc.scalar.copy(qT[:, c0:c0 + 512], ps[0:64, 0:512])
                    nc.vector.tensor_copy(vtmp[64:128, c0:c0 + 512],
                                          ps[64:128, 0:512])
                    nc.vector.tensor_copy(kpack[:, c0:c0 + 512],
                                          ps[0:64, 512:1024])
                    b = ci // 8
                    for t in range(4):
                        jb = (ci % 8) * 4 + t
                        w0 = c0 + t * 128
                        ptr = opool.tile([128, 64], MM_DT, tag="o", name="ptr")
                        nc.tensor.transpose(ptr[:], vtmp[:, w0:w0 + 128],
                                            ident[:, 64:128])
                        nc.vector.tensor_copy(
                            vaug[b][:, jb * 65:jb * 65 + 64], ptr[:])

                # ---- attention ----
                for ci in range(NCH):
                    b = ci // 8
                    i0 = ci * 512
                    ps_oA = opool.tile([65, 512], dt.float32, tag="o", name="ps_oA")
                    ps_oB = opool.tile([65, 512], dt.float32, tag="o", name="ps_oB")
                    acts = [p for p in range(NPAIR) if p not in dve_set]
                    dves = [p for p in range(NPAIR) if p in dve_set]
                    for p in range(NPAIR):
                        j0 = b * N + 2 * p * 128
                        sp = spool.tile([128, 1024], dt.float32, tag="spair", name="sp")
                        nc.tensor.matmul(sp[:, 0:512], kpack[:, j0:j0 + 128],
                                         qT[:, i0:i0 + 512], start=True, stop=True)
                        nc.tensor.matmul(sp[:, 512:1024], kpack[:, j0 + 128:j0 + 256],
                                         qT[:, i0:i0 + 512], start=True, stop=True)
                        pt = pt_p.tile([128, 1024], MM_DT, tag="pt", name="ptile")
                        if p in dve_set:
                            nc.vector.tensor_scalar(
                                pt[:].bitcast(dt.int16), sp[:], C1, C2,
                                mybir.AluOpType.mult, mybir.AluOpType.add)
                        else:
                            nc.scalar.activation(
                                pt[:], sp[:], mybir.ActivationFunctionType.Exp,
                                bias=actbias[:, 0:1], scale=0.125)
                        jb = 2 * p
                        dv = p in dve_set
                        ps_o = ps_oB if dv else ps_oA
                        first = p == (dves[0] if dv else acts[0])
                        last = p == (dves[-1] if dv else acts[-1])
                        nc.tensor.matmul(ps_o[:],
                                         vaug[b][:, jb * 65:jb * 65 + 65],
                                         pt[:, 0:512],
                                         start=first, stop=False)
                        nc.tensor.matmul(ps_o[:],
                                         vaug[b][:, (jb + 1) * 65:(jb + 1) * 65 + 65],
                                         pt[:, 512:1024],
                                         start=False, stop=last)
                    out_t = ot_p.tile([HD + 1, 512], dt.float32, tag="ot", name="out_t")
                    nc.scalar.copy(out_t[:], ps_oA[:])
                    nc.vector.scalar_tensor_tensor(
                        out_t[:], ps_oB[:], 0.0, out_t[:],
                        mybir.AluOpType.add, mybir.AluOpType.add)
                    nc.sync.dma_start(oub[ci], out_t[:])

            if repeat == 1:
                body()
            else:
                with tc.For_i(0, repeat, 1) as _i:
                    body()

    nc.compile()
    return nc


def _make_in_maps(x, w_qkv, w_o, b_o):
    xtp = np.ascontiguousarray(
        x.transpose(2, 1, 0).reshape(D, NB)).astype(MM_NP)
    vs = np.float32(2.0 ** (-G))
    in_maps = []
    for c in range(H):
        wq = w_qkv[c * HD:(c + 1) * HD].T.astype(MM_NP)                    # [512, 64]
        wk = w_qkv[D + c * HD:D + (c + 1) * HD].T.astype(MM_NP)
        wv = (w_qkv[2 * D + c * HD:2 * D + (c + 1) * HD].T * vs).astype(MM_NP)
        wa = np.ascontiguousarray(np.concatenate([wq, wv], 1))             # [512, 128]
        wb = np.ascontiguousarray(np.concatenate([wk, wv], 1))             # [512, 128]
        in_maps.append({"xt": xtp, "wa": wa, "wb": wb})
    return in_maps


_NC_CACHE = {}


def _get_nc(repeat=1, **kw):
    key = (repeat, tuple(sorted(kw.items())))
    if key not in _NC_CACHE:
        _NC_CACHE[key] = _build(repeat=repeat, **kw)
    return _NC_CACHE[key]


def kernel(x, w_qkv, w_o, b_o):
    x = np.asarray(x, np.float32)
    w_qkv = np.asarray(w_qkv, np.float32)
    w_o = np.asarray(w_o, np.float32)
    b_o = np.asarray(b_o, np.float32)
    assert x.shape == (N, B, D), x.shape
    nc = _get_nc()
    in_maps = _make_in_maps(x, w_qkv, w_o, b_o)
    res = run_bass_kernel_spmd(nc, in_maps, list(range(8)))
    acc = np.zeros((B, N, D), np.float64)
    for c, r in enumerate(res.results):
        O = np.asarray(r["oub"], np.float32).reshape(B, 8, HD + 1, 512)
        woc = w_o[:, c * HD:(c + 1) * HD]                       # [512, 64]
        for b in range(B):
            ouT = O[b, :, 0:HD, :].transpose(0, 2, 1).reshape(N, HD)
            l = O[b, :, HD, :].reshape(N, 1)
            acc[b] += (ouT @ woc.T) / l
    return (acc + b_o[None, None, :]).astype(np.float32)
